# revision 14
# baseline (speedup 1.0000x reference)
"""DeltaLoss kernel for 8 TRN2 NeuronCores (Bass/Tile).

Problem: loss = 0.5*(CE_row + CE_col) over mma = 100 * unit(td) @ unit(im).T
where td/im are all ordered-pair deltas of txtf/imgf [96, 512] -> P = 9120.

Algebraic reduction: mma[p,q] = 100 * (pairA @ G @ pairA.T)[p,q] / (ntd[p]*nim[q])
with G = txtf @ imgf.T only [96, 96], pairA the +/-1 pair-difference matrix.
Each core computes a 1140-column block of the transposed logits matrix
AT[q, p] (q = all 9120 on partition tiles, p = core's slice on free dim),
applies exp with a fixed safe shift (columns come in +/- pairs so every
row/col max is in [0, 100]; actual maxes ~[10, 24]), and reduces:
  - accum_out of the exp activation -> partial column sums (free-dim sums)
  - bf16 running accumulator + final ones-matmul -> row sums
Host combines: lse = SHIFT + log(sums), loss = mean(lse_row+lse_col)/2 - mean(diag).
"""

import os
import sys

import numpy as np

sys.path.insert(0, "/opt/trn_rl_repo")

N = 96
D = 512
P = N * (N - 1)  # 9120
NCORES = 8
PSLICE = P // NCORES  # 1140
QT = (P + 127) // 128  # 72 q-tiles; last tile has only 32 rows
QTAIL = P - (QT - 1) * 128  # 32
SHIFT = 30.0
KCH = 128  # contraction chunk for the [96,96] gram matmuls (D=512 -> 4 chunks)

_CACHE = {}


def _pair_constants():
    i, j = np.meshgrid(np.arange(N), np.arange(N), indexing="ij")
    mask = i != j
    ii, jj = i[mask], j[mask]
    pairA = np.zeros((P, N), np.float32)
    pairA[np.arange(P), ii] = 1.0
    pairA[np.arange(P), jj] = -1.0
    return np.ascontiguousarray(pairA.T)  # pairAT [N, P]


def _build():
    import concourse.bass as bass
    import concourse.tile as tile
    from concourse import bacc, mybir

    f32 = mybir.dt.float32
    f32r = mybir.dt.float32r
    bf16 = mybir.dt.bfloat16
    AF = mybir.ActivationFunctionType
    ALU = mybir.AluOpType

    nc = bacc.Bacc("TRN2", target_bir_lowering=False, debug=False,
                   num_devices=NCORES)

    # DRAM I/O (per-core shards supplied via in_maps)
    d_txtfT = nc.dram_tensor("txtfT", [D, N], f32, kind="ExternalInput").ap()
    d_imgfT = nc.dram_tensor("imgfT", [D, N], f32, kind="ExternalInput").ap()
    d_pat = nc.dram_tensor("pat", [N, P], f32r, kind="ExternalInput").ap()
    d_pslice = nc.dram_tensor("pslice", [N, PSLICE], f32r,
                              kind="ExternalInput").ap()
    d_diag = nc.dram_tensor("diag_o", [1, PSLICE], f32,
                            kind="ExternalOutput").ap()
    d_rowsum = nc.dram_tensor("rowsum_o", [1, PSLICE], f32,
                              kind="ExternalOutput").ap()
    d_colsum = nc.dram_tensor("colsum_o", [128, QT], f32,
                              kind="ExternalOutput").ap()

    chunks = [(0, 512), (512, 1024), (1024, PSLICE)]

    with tile.TileContext(nc) as tc:
        with tc.tile_pool(name="persist", bufs=1) as persist, \
             tc.tile_pool(name="pconst", bufs=1) as pconst:

            # ---- load inputs (img + pat chunks first: they gate the
            # invnim_col critical path that the exp loop waits on) ----
            img_sb = pconst.tile([KCH, D // KCH, N], f32)
            nc.sync.dma_start(out=img_sb,
                              in_=d_imgfT.rearrange("(a p) c -> p a c", p=KCH))
            psl_sb = persist.tile([N, PSLICE], f32r)
            nc.sync.dma_start(out=psl_sb, in_=d_pslice)
            txt_sb = pconst.tile([KCH, D // KCH, N], f32)
            nc.sync.dma_start(out=txt_sb,
                              in_=d_txtfT.rearrange("(a p) c -> p a c", p=KCH))
            pat_sb = persist.tile([N, P], f32r)
            DCH = 1024
            for c0 in range(0, P, DCH):
                c1 = min(c0 + DCH, P)
                nc.sync.dma_start(out=pat_sb[:, c0:c1], in_=d_pat[:, c0:c1])

            # memset cannot write f32r; round via DVE copy (values exact)
            ones96f = pconst.tile([N, 2], f32)
            nc.vector.memset(ones96f, 1.0)
            ones96_2 = pconst.tile([N, 2], f32r)
            nc.vector.tensor_copy(ones96_2, ones96f)
            ones96 = ones96_2[:, 0:1]
            hundredsf = pconst.tile([1, N], f32)
            nc.vector.memset(hundredsf, 100.0)
            hundreds = pconst.tile([1, N], f32r)
            nc.vector.tensor_copy(hundreds, hundredsf)
            zeros128 = pconst.tile([128, 1], f32)
            nc.vector.memset(zeros128, 0.0)

            # ---- gram matrices G = txtf@imgf.T, Tt, Ti (fp32) ----
            with tc.tile_pool(name="gpsum", bufs=2, space="PSUM") as gpsum:
                def gram(lhs, rhs, tag):
                    ps = gpsum.tile([N, N], f32, tag="gram_ps")
                    for a in range(D // KCH):
                        nc.tensor.matmul(ps, lhsT=lhs[:, a, :],
                                         rhs=rhs[:, a, :],
                                         start=(a == 0),
                                         stop=(a == D // KCH - 1))
                    sb = pconst.tile([N, N], f32r, tag=tag)
                    nc.vector.tensor_copy(sb, ps)
                    return sb

                Ti_sb = gram(img_sb, img_sb, "Ti")
                Tt_sb = gram(txt_sb, txt_sb, "Tt")
                G_sb = gram(txt_sb, img_sb, "G")

            patr = pat_sb
            pat_f32 = pat_sb.bitcast(f32)
            pslr = psl_sb
            psl_f32 = psl_sb.bitcast(f32)
            Tir = Ti_sb
            Ttr = Tt_sb
            Gr = G_sb
            ones96r = ones96

            def rsqrt_newton(dst, v, pool, tag):
                # dst = 1/sqrt(v); ACT Sqrt (~0.4% worst) polished by one
                # Newton step on DVE.
                s = pool.tile(v.shape, f32, tag=f"{tag}_s")
                nc.scalar.activation(s, v, AF.Sqrt, bias=zeros128[:v.shape[0]])
                r = pool.tile(v.shape, f32, tag=f"{tag}_r")
                nc.vector.reciprocal(r, s)
                t1 = pool.tile(v.shape, f32, tag=f"{tag}_t1")
                nc.vector.tensor_mul(t1, r, r)
                nc.vector.tensor_mul(t1, t1, v)
                nc.vector.tensor_scalar(t1, t1, -0.5, 1.5, ALU.mult, ALU.add)
                nc.vector.tensor_mul(dst, r, t1)

            invnim_col = pconst.tile([128, QT], f32)
            HsT_sb = persist.tile([N, PSLICE], f32r)
            diag_sb = pconst.tile([1, PSLICE], f32)

            # ---- prep phase ----
            with tc.tile_pool(name="bigtmp", bufs=1) as bigtmp, \
                 tc.tile_pool(name="cpsum", bufs=3, space="PSUM") as cpsum, \
                 tc.tile_pool(name="wpsum", bufs=1, space="PSUM") as wpsum:
                # nim2 (full, col layout):
                # prod_full[c,q] = (Ti @ pat)[c,q] * pat[c,q]; nim2 = colsums
                prod_full = bigtmp.tile([N, P], f32r)
                prodr = prod_full
                nim2_ps = cpsum.tile([128, 2 * QT], f32, tag="sm")
                for c0 in range(0, P, 512):
                    c1 = min(c0 + 512, P)
                    psc = cpsum.tile([N, 512], f32, tag="sm")
                    nc.tensor.matmul(psc[:, :c1 - c0], lhsT=Tir,
                                     rhs=patr[:, c0:c1], start=True, stop=True)
                    nc.vector.tensor_mul(prod_full[:, c0:c1], psc[:, :c1 - c0],
                                         pat_f32[:, c0:c1])
                    for t in range(c0 // 128, (c1 + 127) // 128):
                        h = min(128, c1 - t * 128)
                        nc.tensor.matmul(nim2_ps[:h, 2 * t:2 * t + 2],
                                         lhsT=prodr[:, t * 128:t * 128 + h],
                                         rhs=ones96_2, start=True, stop=True)
                v_col = pconst.tile([128, QT], f32)
                nc.vector.tensor_copy(
                    v_col, nim2_ps.rearrange("p (t two) -> p t two", two=2)[:, :, 0])
                rsqrt_newton(invnim_col, v_col, pconst, "nimcol")

                # slice-local: ntd2 and nim2[slice] (free layout)
                def sandwich_cols(gram_r, out_prod_tag):
                    # sbuf [N, PSLICE] prod = (gram @ pslice) * pslice
                    pr = bigtmp.tile([N, PSLICE], f32r, tag=out_prod_tag)
                    for c0, c1 in chunks:
                        psc = cpsum.tile([N, 512], f32, tag="sm")
                        nc.tensor.matmul(psc[:, :c1 - c0], lhsT=gram_r,
                                         rhs=pslr[:, c0:c1], start=True,
                                         stop=True)
                        nc.vector.tensor_mul(pr[:, c0:c1], psc[:, :c1 - c0],
                                             psl_f32[:, c0:c1])
                    return pr

                prod_t = sandwich_cols(Ttr, "prod_t")
                prod_i = sandwich_cols(Tir, "prod_i")

                # ntd2 in [0:PSLICE], nim2[slice] in [PSLICE:2*PSLICE],
                # side by side on one partition (partition bases must be
                # 32-aligned, so a [2, PSLICE] stack is not writable).
                stack2 = pconst.tile([1, 2 * PSLICE], f32)
                for row, pr in ((0, prod_t), (1, prod_i)):
                    for c0, c1 in chunks:
                        psn = cpsum.tile([1, 512], f32, tag="sm")
                        nc.tensor.matmul(psn[:, :c1 - c0], lhsT=ones96r,
                                         rhs=pr[:, c0:c1], start=True,
                                         stop=True)
                        nc.vector.tensor_copy(
                            stack2[0:1, row * PSLICE + c0:row * PSLICE + c1],
                            psn[:, :c1 - c0])

                inv2 = pconst.tile([1, 2 * PSLICE], f32r)
                rsqrt_newton(inv2, stack2, pconst, "inv2")
                inv_ntd = inv2[0:1, 0:PSLICE]
                inv_nim_sl = inv2.bitcast(f32)[0:1, PSLICE:2 * PSLICE]

                # HsT = (G.T @ pairAT_slice) * (100/ntd[p])
                bc_ps = wpsum.tile([N, PSLICE], f32, tag="wide")
                for c0, c1 in chunks:
                    nc.tensor.matmul(bc_ps[:, c0:c1],
                                     lhsT=hundreds,
                                     rhs=inv_ntd[:, c0:c1],
                                     start=True, stop=True)
                bc_sb = pconst.tile([N, PSLICE], f32)
                nc.scalar.copy(bc_sb, bc_ps)

                psH = wpsum.tile([N, PSLICE], f32, tag="wide")
                for c0, c1 in chunks:
                    nc.tensor.matmul(psH[:, c0:c1], lhsT=Gr,
                                     rhs=pslr[:, c0:c1], start=True, stop=True)
                nc.vector.tensor_mul(HsT_sb, psH, bc_sb)

                # diag = (HrawT . pat_slice colsums) * 100/ntd * 1/nim
                prod_d = bigtmp.tile([N, PSLICE], f32r, tag="prod_d")
                nc.vector.tensor_mul(prod_d, psH, psl_f32)
                diag_ps = wpsum.tile([1, PSLICE], f32, tag="wide")
                for c0, c1 in chunks:
                    nc.tensor.matmul(diag_ps[:, c0:c1], lhsT=ones96r,
                                     rhs=prod_d[:, c0:c1],
                                     start=True, stop=True)
                nc.vector.tensor_mul(diag_sb, diag_ps, bc_sb[0:1, :])
                nc.vector.tensor_mul(diag_sb, diag_sb, inv_nim_sl)
                nc.sync.dma_start(out=d_diag, in_=diag_sb)

            # ---- main loop over 72 q-tiles ----
            HsTr = HsT_sb
            acc = persist.tile([128, PSLICE], bf16)
            nc.vector.memset(acc, 0.0)
            colsum_sb = persist.tile([128, QT], f32)
            negshift = pconst.tile([128, 1], f32)
            nc.vector.memset(negshift, -SHIFT)

            with tc.tile_pool(name="mpsum", bufs=2, space="PSUM") as mpsum, \
                 tc.tile_pool(name="epool", bufs=3) as epool:
                for t in range(QT):
                    h = 128 if t < QT - 1 else QTAIL
                    ps = mpsum.tile([128, PSLICE], f32, tag="logits")
                    for c0, c1 in chunks:
                        nc.tensor.matmul(ps[:h, c0:c1],
                                         lhsT=patr[:, t * 128:t * 128 + h],
                                         rhs=HsTr[:, c0:c1],
                                         start=True, stop=True)
                    e = epool.tile([128, PSLICE], bf16, tag="exp")
                    nc.scalar.activation(e[:h], ps[:h], AF.Exp,
                                         bias=negshift[:h],
                                         scale=invnim_col[:h, t:t + 1],
                                         accum_out=colsum_sb[:h, t:t + 1])
                    nc.vector.tensor_add(acc[:h], acc[:h], e[:h])

                # rowsum = partition-reduce of acc
                ones128 = pconst.tile([128, 1], bf16)
                nc.vector.memset(ones128, 1.0)
                rowsum_sb = pconst.tile([1, PSLICE], f32)
                for c0, c1 in chunks:
                    rs_ps = mpsum.tile([1, 512], f32, tag="rs")
                    nc.tensor.matmul(rs_ps[:, :c1 - c0], lhsT=ones128,
                                     rhs=acc[:, c0:c1], start=True, stop=True)
                    nc.scalar.copy(rowsum_sb[:, c0:c1], rs_ps[:, :c1 - c0])
                nc.sync.dma_start(out=d_rowsum, in_=rowsum_sb)
                nc.sync.dma_start(out=d_colsum, in_=colsum_sb)

    nc.compile()
    return nc


def _get_nc():
    if "nc" not in _CACHE:
        _CACHE["nc"] = _build()
        _CACHE["pairAT"] = _pair_constants()
    return _CACHE["nc"], _CACHE["pairAT"]


def kernel(txtf: np.ndarray, imgf: np.ndarray) -> np.ndarray:
    from concourse import bass_utils

    nc, pairAT = _get_nc()
    txtf = np.asarray(txtf, np.float32)
    imgf = np.asarray(imgf, np.float32)
    txtfT = np.ascontiguousarray(txtf.T)
    imgfT = np.ascontiguousarray(imgf.T)

    in_maps = []
    for c in range(NCORES):
        sl = pairAT[:, c * PSLICE:(c + 1) * PSLICE]
        in_maps.append({
            "txtfT": txtfT,
            "imgfT": imgfT,
            "pat": pairAT,
            "pslice": np.ascontiguousarray(sl),
        })

    res = bass_utils.run_bass_kernel_spmd(
        nc, in_maps, core_ids=list(range(NCORES)))
    outs = res.results

    diag = np.concatenate([outs[c]["diag_o"][0] for c in range(NCORES)])
    rowsum = np.concatenate([outs[c]["rowsum_o"][0] for c in range(NCORES)])
    # colsum col-layout: [128, QT], q = t*128 + part (tail tile only QTAIL)
    colsum = np.zeros(P, np.float64)
    for c in range(NCORES):
        a = outs[c]["colsum_o"].astype(np.float64)
        colsum[:(QT - 1) * 128] += a[:, :QT - 1].T.reshape(-1)
        colsum[(QT - 1) * 128:] += a[:QTAIL, QT - 1]

    lse_row = SHIFT + np.log(rowsum.astype(np.float64))
    lse_col = SHIFT + np.log(colsum)
    loss1 = np.mean(lse_row - diag)
    loss2 = np.mean(lse_col - diag)
    return np.float32(0.5 * (loss1 + loss2))
